# revision 1
# baseline (speedup 1.0000x reference)
"""Trainium2 Bass kernel: 3D factorized-position attention (dense_transformer).

Reference computation (per batch b of 8):
    x = fmap[b].reshape(256, 1568)                       # channels x positions
    qkv = W_qkv @ x ; q,k,v heads of dim 128, 4 heads
    emb[n,128] = pos_f+pos_h+pos_w broadcast-sum
    sim = (q*scale) @ k^T + (q*scale) @ emb^T  ==  qs @ (k+embT)^T
    out = softmax(sim) @ v, reassembled to (512, 8, 14, 14)

Sharding: batch (8) across the 8 NeuronCores, zero collectives.

Engine balance (per core):
    ACT  : exp only, 104 insts x stripe width              ~88us  <- limiter
    PE   : proj 38.4K + S 81.5K + U 81.5K cycles           ~85us
    DVE  : E add-tree for l, k'=k+emb, emb build, recip    ~55us
    Pool : PSUM->SBUF evacuations, l all-reduce, norm mul  ~54us
Key structures:
  - All input DMAs are consolidated into 7 transfers (each DMA costs ~625ns
    on the single HWDGE queue) with host-side layouts ordered so the
    critical-path slices (head-0 k/q weights, first x columns) land first;
    the positional embedding is built on-device from a 4KB pos table.
  - Projection work is interleaved into the attention unit stream as
    "pieces" (v^T per (head, j-tile), k'/q per n-chunk) sized so each
    head's PE work just undercuts ACT's exp time and ACT never drains.
  - l = row-sums of E via a binary add-tree on DVE (bf16 2x) finished by a
    gpsimd partition_all_reduce; the whole normalize chain (all-reduce ->
    reciprocal -> multiply -> DMA) avoids PE so head boundaries never block
    the in-order matmul queue.
"""

import numpy as np
import ml_dtypes

# --- hardcoded problem shapes (self-contained: no spec.json / reference.py) ---
B = 8
C = 256          # input channels
F, HH, WW = 8, 14, 14
N = F * HH * WW  # 1568 positions
HEADS = 4
D = 128          # head dim
SCALE = D ** -0.5
N_CORES = 8

CC = C // 128            # c chunks (2)
NJ = (N + 127) // 128    # j tiles (13; last is 32 wide)
STRIPES = [(0, 1024), (1024, 544)]   # i stripes (psum-bank pairs)
# wqk block order (head-0 slices first so the prologue's DMA is small)
BLK = [("k", 0), ("q", 0), ("k", 1), ("q", 1), ("k", 2), ("q", 2),
       ("k", 3), ("q", 3)]
BLK_IDX = {bq: i for i, bq in enumerate(BLK)}
XSPL = [(0, 512), (512, 512), (1024, N - 1024)]


def _chunks(start, width, bank=512):
    """Split [start, start+width) into psum-bank-aligned chunks (<=512 each)."""
    out = []
    c = 0
    while c < width:
        w = min(bank - ((start + c) % bank), width - c)
        out.append((c, w))
        c += w
    return out


_CACHE = {}
INST_LABELS = {}


def _lab(bi, label):
    INST_LABELS[bi.ins.name] = label
    return bi


def _build(reps=1):
    key = ("nc", reps)
    if key in _CACHE:
        return _CACHE[key]

    import concourse.bacc as bacc
    import concourse.tile as tile
    import concourse.mybir as mybir
    import concourse.bass_isa as bass_isa

    f32 = mybir.dt.float32
    bf16 = mybir.dt.bfloat16
    AF = mybir.ActivationFunctionType

    nc = bacc.Bacc("TRN2", target_bir_lowering=False, debug=False,
                   enable_asserts=False, num_devices=N_CORES)

    x_d = nc.declare_dram_parameter("x", [128, CC * N], bf16, isOutput=False)
    wqk_d = nc.declare_dram_parameter("wqk", [128, CC * 1024], bf16, isOutput=False)
    wv_d = nc.declare_dram_parameter("wv", [128, CC * 512], bf16, isOutput=False)
    pos_d = nc.declare_dram_parameter("pos", [128, F + HH + WW], f32,
                                      isOutput=False)
    out_d = nc.declare_dram_parameter("out", [HEADS * D, N], f32, isOutput=True)

    import contextlib
    with tile.TileContext(nc) as tc:
        rep_loop = tc.For_i(0, reps, 1) if reps > 1 else contextlib.nullcontext()
        with rep_loop:
          with (
            tc.tile_pool(name="const", bufs=1) as constp,
            tc.tile_pool(name="epool", bufs=8) as epool,
            tc.tile_pool(name="treep", bufs=8) as treep,
            tc.tile_pool(name="lrp", bufs=2) as lrp,
            tc.tile_pool(name="outp", bufs=2) as outp,
            tc.tile_pool(name="rbp", bufs=2) as rbp,
            tc.tile_pool(name="ps_s", bufs=2, space="PSUM") as ps_s,
            tc.tile_pool(name="acc", bufs=4, space="PSUM") as accp,
        ):
            # ---- input DMAs on TWO hwdge rings (SP + ACT), critical first ----
            pos_sb = constp.tile([128, F + HH + WW], f32, tag="pos")
            nc.sync.dma_start(pos_sb[:, :], pos_d.ap())
            x_spl = []
            for xi, (xc0, xcw) in enumerate(XSPL):
                xt = constp.tile([128, CC * xcw], bf16, tag=f"xs{xi}",
                                 name=f"xs{xi}")
                x_spl.append(xt)
            nc.scalar.dma_start(x_spl[0][:, :], x_d.ap()[:, 0:1024])
            wqk_sb = constp.tile([128, CC * 1024], bf16, tag="wqk")
            nc.sync.dma_start(wqk_sb[:, 0:512], wqk_d.ap()[:, 0:512])
            nc.scalar.dma_start(x_spl[1][:, :], x_d.ap()[:, 1024:2048])
            wv_sb = constp.tile([128, CC * 512], bf16, tag="wv")
            nc.sync.dma_start(wv_sb[:, :], wv_d.ap())
            nc.scalar.dma_start(x_spl[2][:, :], x_d.ap()[:, 2048:CC * N])
            nc.sync.dma_start(wqk_sb[:, 512:CC * 1024],
                              wqk_d.ap()[:, 512:CC * 1024])

            # ---- PE clock warmup: dummy matmuls while DMAs land (the PE
            # runs at 0.65/1.2 GHz until it has been busy ~3us) ----
            dum_sb = constp.tile([128, 128], bf16, tag="dum")
            nc.vector.memset(dum_sb[:, :], 0.0)
            dum_ps = accp.tile([128, 512], f32, tag="acc", name="dum_ps")
            for _ in range(32):
                _lab(nc.tensor.matmul(dum_ps[:, 0:128], lhsT=dum_sb[:, :],
                                      rhs=dum_sb[:, :], start=True, stop=True),
                     "warmup-mm")

            def x_slice(cc, c0, cw):
                for xi, (xc0, xcw) in enumerate(XSPL):
                    if xc0 <= c0 and c0 + cw <= xc0 + xcw:
                        return x_spl[xi][:, cc * xcw + c0 - xc0:
                                         cc * xcw + c0 - xc0 + cw]
                raise AssertionError((c0, cw))

            def wqk_slice(dest, h, cc):
                b = BLK_IDX[(dest, h)] * 256 + cc * 128
                return wqk_sb[:, b: b + 128]

            # ---- on-device positional embedding: embT[d, (f h w)],
            # emitted in fh-chunks so k' chunk 0 unblocks early ----
            fh_sb = constp.tile([128, F * HH], f32, tag="fh")
            for f in range(F):
                nc.vector.tensor_add(
                    fh_sb[:, f * HH:(f + 1) * HH], pos_sb[:, F:F + HH],
                    pos_sb[:, f:f + 1].broadcast_to((128, HH)))
            emb_sb = constp.tile([128, N], f32, tag="emb")
            for a0, a1 in ((0, 37), (37, 74), (74, F * HH)):
                nc.vector.tensor_add(
                    emb_sb[:, a0 * WW:a1 * WW].rearrange(
                        "p (a b) -> p a b", b=WW),
                    fh_sb[:, a0:a1].unsqueeze(2)
                    .broadcast_to((128, a1 - a0, WW)),
                    pos_sb[:, F + HH:F + HH + WW].unsqueeze(1)
                    .broadcast_to((128, a1 - a0, WW)))

            warm = constp.tile([1, 8], bf16, tag="warm")
            nc.vector.memset(warm[0:1, :], 0.0)
            nc.scalar.activation(warm[0:1, 4:5], warm[0:1, 0:1], AF.Exp)
            ones_sb = constp.tile([128, 128], bf16, tag="ones")
            nc.vector.memset(ones_sb[:, :], 1.0)

            q_sb = constp.tile([128, HEADS * N], bf16, tag="q")
            kp_sb = constp.tile([128, HEADS * N], bf16, tag="kp")
            vt_sb = constp.tile([128, NJ * 512], bf16, tag="vt")

            PROJ_CHUNKS = _chunks(0, N)  # (0,512),(512,512),(1024,512),(1536,32)

            def emit_kq_piece(dest, h, ci, evac="pool", split_add=False):
                """Project one n-chunk of q or k' for head h into SBUF."""
                c0, cw = PROJ_CHUNKS[ci]
                pq = accp.tile([128, 512], f32, tag="acc", name="pq")
                for cc in range(CC):
                    _lab(nc.tensor.matmul(
                        pq[:, 0:cw],
                        lhsT=wqk_slice(dest, h, cc),
                        rhs=x_slice(cc, c0, cw),
                        start=(cc == 0), stop=(cc == CC - 1),
                    ), f"proj-{dest}{h}c{ci}")
                if dest == "q":
                    dst = q_sb[:, h * N + c0: h * N + c0 + cw]
                    if evac == "act":   # prologue only: ACT is idle there
                        nc.scalar.copy(dst, pq[:, 0:cw])
                    else:               # gpsimd can't read PSUM: DVE
                        nc.vector.tensor_copy(dst, pq[:, 0:cw])
                else:
                    # split_add: first j-tile's 128 cols land first so the
                    # first S matmul unblocks early (prologue only)
                    parts = ((0, 128), (128, cw - 128)) if split_add else ((0, cw),)
                    for p0, pw_ in parts:
                        nc.vector.tensor_add(
                            kp_sb[:, h * N + c0 + p0: h * N + c0 + p0 + pw_],
                            pq[:, p0:p0 + pw_],
                            emb_sb[:, c0 + p0: c0 + p0 + pw_])

            vt_groups = {}

            def emit_vt_mm(h, jt, slot):
                """One v^T projection matmul into the group's shared psum."""
                jw = min(128, N - jt * 128)
                if slot == 0:
                    vt_groups[(h, jt // 4)] = accp.tile(
                        [128, 512], f32, tag="acc", name="pv")
                pv = vt_groups[(h, jt // 4)]
                for cc in range(CC):
                    _lab(nc.tensor.matmul(
                        pv[0:jw, slot * 128: slot * 128 + 128],
                        lhsT=x_slice(cc, jt * 128, jw),
                        rhs=wv_sb[:, cc * 512 + h * 128: cc * 512 + (h + 1) * 128],
                        start=(cc == 0), stop=(cc == CC - 1),
                    ), f"proj-vt{h}_{jt}")

            def emit_vt_copy(h, jt0, nj):
                """One strided DVE copy evacuates nj v^T tiles at once."""
                pv = vt_groups.pop((h, jt0 // 4))
                jw = min(128, N - (jt0 + nj - 1) * 128)
                if nj == 1:
                    nc.vector.tensor_copy(
                        vt_sb[0:jw, jt0 * 512 + h * 128: jt0 * 512 + h * 128 + 128],
                        pv[0:jw, 0:128])
                else:
                    dst = vt_sb[:, :].rearrange("p (j c) -> p j c", c=512)[
                        0:128, jt0:jt0 + nj, h * 128:(h + 1) * 128]
                    src = pv[0:128, 0:nj * 128].rearrange(
                        "p (j c) -> p j c", c=128)
                    nc.vector.tensor_copy(dst, src)

            def emit_piece(p):
                if p[0] == "vtm":
                    emit_vt_mm(p[1], p[2], p[3])
                elif p[0] == "vtc":
                    emit_vt_copy(p[1], p[2], p[3])
                else:
                    emit_kq_piece(p[1], p[2], p[3])

            # ---- prologue: just enough for S(h0,s0,jt0): k' chunk 0 and
            # the full q stripe-0 (k' c1 / v^T follow as unit-0 pieces) ----
            emit_kq_piece("q", 0, 0, evac="act")
            emit_kq_piece("k", 0, 0, split_add=True)
            emit_kq_piece("q", 0, 1, evac="act")
            for jt in range(4):
                emit_vt_mm(0, jt, jt)
            emit_vt_copy(0, 0, 4)

            # ---- piece schedule (unit granularity, 26 units per head).
            # Units b+3..5 and b+16..20 are piece-free: that's where the
            # previous stripe's pu slots are still live alongside this
            # stripe's, so a piece allocating PSUM there would stall PE on
            # the normalize chain. ----
            pieces_at = {}

            def add_piece(u, p):
                pieces_at.setdefault(u, []).append(p)

            for h in range(HEADS):
                b = h * 26
                if h == 0:
                    # no prior stripe: front-load everything
                    add_piece(0, ("kq", "k", 0, 1))
                    add_piece(1, ("vtm", 0, 4, 0))
                    add_piece(1, ("vtm", 0, 5, 1))
                    add_piece(2, ("vtm", 0, 6, 2))
                    add_piece(2, ("vtm", 0, 7, 3))
                    add_piece(2, ("vtc", 0, 4, 4))
                    add_piece(3, ("vtm", 0, 8, 0))
                    add_piece(3, ("vtm", 0, 9, 1))
                    add_piece(4, ("vtm", 0, 10, 2))
                    add_piece(4, ("vtm", 0, 11, 3))
                    add_piece(4, ("vtc", 0, 8, 4))
                    add_piece(5, ("vtm", 0, 12, 0))
                    add_piece(5, ("vtc", 0, 12, 1))
                    add_piece(5, ("kq", "k", 0, 2))  # feeds S jt8 at u8
                    add_piece(6, ("kq", "k", 0, 3))
                    add_piece(7, ("kq", "q", 0, 2))
                    add_piece(8, ("kq", "q", 0, 3))
                    add_piece(9, ("kq", "k", 1, 0))
                    add_piece(10, ("kq", "k", 1, 1))
                    add_piece(11, ("kq", "q", 1, 0))
                    add_piece(12, ("kq", "q", 1, 1))
                    add_piece(19, ("kq", "k", 1, 2))
                    add_piece(20, ("kq", "k", 1, 3))
                    add_piece(21, ("kq", "q", 1, 2))
                    add_piece(22, ("kq", "q", 1, 3))
                else:
                    # units b+0..4 are the prev-s1 normalize window: no pieces
                    add_piece(b + 5, ("vtm", h, 4, 0))
                    add_piece(b + 5, ("vtm", h, 5, 1))
                    add_piece(b + 6, ("vtm", h, 6, 2))
                    add_piece(b + 6, ("vtm", h, 7, 3))
                    add_piece(b + 6, ("vtc", h, 4, 4))
                    add_piece(b + 7, ("vtm", h, 8, 0))
                    add_piece(b + 7, ("vtm", h, 9, 1))
                    add_piece(b + 8, ("vtm", h, 10, 2))
                    add_piece(b + 8, ("vtm", h, 11, 3))
                    add_piece(b + 8, ("vtc", h, 8, 4))
                    add_piece(b + 9, ("vtm", h, 12, 0))
                    add_piece(b + 9, ("vtc", h, 12, 1))
                    if h + 1 < HEADS:
                        add_piece(b + 10, ("kq", "k", h + 1, 0))
                        add_piece(b + 11, ("kq", "k", h + 1, 1))
                        add_piece(b + 12, ("kq", "q", h + 1, 0))
                        add_piece(b + 19, ("kq", "q", h + 1, 1))
                        add_piece(b + 20, ("kq", "k", h + 1, 2))
                        add_piece(b + 21, ("kq", "k", h + 1, 3))
                        add_piece(b + 22, ("kq", "q", h + 1, 2))
                        add_piece(b + 23, ("kq", "q", h + 1, 3))
                if h + 1 < HEADS:
                    # units b+13..15 (stripe-1 start) and b+16..18 (s0
                    # normalize window) stay piece-free
                    add_piece(b + 24, ("vtm", h + 1, 0, 0))
                    add_piece(b + 24, ("vtm", h + 1, 1, 1))
                    add_piece(b + 25, ("vtm", h + 1, 2, 2))
                    add_piece(b + 25, ("vtm", h + 1, 3, 3))
                    add_piece(b + 25, ("vtc", h + 1, 0, 4))

            # ---- attention unit stream ----
            def emit_unit_s(st, jt):
                h, s0, sw = st["h"], st["s0"], st["sw"]
                jw = min(128, N - jt * 128)
                ps = ps_s.tile([128, 1024], f32, tag="ps_s")
                for c0, cw in st["cks"]:
                    _lab(nc.tensor.matmul(
                        ps[0:jw, c0:c0 + cw],
                        lhsT=kp_sb[:, h * N + jt * 128: h * N + jt * 128 + jw],
                        rhs=q_sb[:, h * N + s0 + c0: h * N + s0 + c0 + cw],
                        start=True, stop=True,
                    ), f"s-mm h{h} s{s0} jt{jt}")
                et = epool.tile([128, 1024], bf16, tag="e")
                _lab(nc.scalar.activation(et[0:jw, 0:sw], ps[0:jw, 0:sw],
                                          AF.Exp), f"exp h{h} s{s0} jt{jt}")
                return et

            def tree_add(st, a, b, eng=None):
                s = treep.tile([128, 1024], bf16, tag="tree", name="tn")
                _lab((eng or nc.vector).tensor_add(
                    s[:, 0:st["sw"]], a[:, 0:st["sw"]], b[:, 0:st["sw"]]),
                    f"tree h{st['h']}")
                return s

            def emit_consumers(st, jt, et):
                h, cks, pu = st["h"], st["cks"], st["pu"]
                jw = min(128, N - jt * 128)
                for ci, (c0, cw) in enumerate(cks):
                    if ci not in pu:
                        pu[ci] = accp.tile([128, 512], f32, tag="acc",
                                           name="pu")
                    _lab(nc.tensor.matmul(
                        pu[ci][:, 0:cw],
                        lhsT=vt_sb[0:jw, jt * 512 + h * 128:
                                   jt * 512 + h * 128 + 128],
                        rhs=et[0:jw, c0:c0 + cw],
                        start=(jt == 0), stop=(jt == NJ - 1),
                    ), f"u-mm h{h} jt{jt}")
                nodes = st["nodes"]
                if jt < NJ - 1:
                    nodes.append((0, et))
                    while len(nodes) >= 2 and nodes[-1][0] == nodes[-2][0]:
                        lv, a = nodes.pop()
                        _, b2 = nodes.pop()
                        nodes.append((lv + 1, tree_add(st, b2, a)))
                else:
                    while len(nodes) >= 2:       # force-merge the remainder
                        _, a = nodes.pop()
                        _, b2 = nodes.pop()
                        nodes.append((9, tree_add(st, b2, a)))
                    t = nodes.pop()[1]
                    # fold the 32-row last tile in-place on low partitions
                    _lab(nc.vector.tensor_add(t[0:jw, 0:st["sw"]],
                                              t[0:jw, 0:st["sw"]],
                                              et[0:jw, 0:st["sw"]]),
                         f"tree-last h{h}")
                    st["tsum"] = t

            def emit_normalize(st, final=False):
                h, s0, sw, t, pu = st["h"], st["s0"], st["sw"], st["tsum"], st["pu"]
                ot_sb = outp.tile([128, 1024], f32, tag="o", name="ot_sb")
                for ci, (c0, cw) in enumerate(st["cks"]):
                    rb = rbp.tile([128, 1024], f32, tag="rb", name="rb")
                    if final:
                        # PE is idle at the very end: a ones-matmul beats the
                        # gpsimd all-reduce on the drain critical path
                        pl = accp.tile([128, 512], f32, tag="acc", name="pl")
                        _lab(nc.tensor.matmul(
                            pl[:, 0:cw], lhsT=ones_sb[0:128, 0:128],
                            rhs=t[0:128, c0:c0 + cw], start=True, stop=True,
                        ), f"l-mm h{h} s{s0}")
                        nc.vector.reciprocal_approx_fast(rb[:, 0:cw],
                                                         pl[:, 0:cw])
                    else:
                        lr = lrp.tile([128, 1024], f32, tag="lr", name="lr")
                        _lab(nc.gpsimd.partition_all_reduce(
                            lr[:, 0:cw], t[:, c0:c0 + cw], channels=128,
                            reduce_op=bass_isa.ReduceOp.add), f"l-red h{h} s{s0}")
                        nc.vector.reciprocal_approx_fast(rb[:, 0:cw],
                                                         lr[:, 0:cw])
                    _lab(nc.vector.tensor_mul(ot_sb[:, c0:c0 + cw],
                                              pu[ci][:, 0:cw],
                                              rb[:, 0:cw]),
                         f"norm-mul h{h} s{s0}")
                    if final:
                        nc.sync.dma_start(
                            out_d.ap()[h * D:(h + 1) * D,
                                       s0 + c0:s0 + c0 + cw],
                            ot_sb[:, c0:c0 + cw])
                if not final:
                    nc.sync.dma_start(
                        out_d.ap()[h * D:(h + 1) * D, s0:s0 + sw],
                        ot_sb[:, 0:sw])

            DEPTH = 3
            units = []
            for h in range(HEADS):
                for s0, sw in STRIPES:
                    st = {"h": h, "s0": s0, "sw": sw, "cks": _chunks(s0, sw),
                          "pu": {}, "nodes": [], "tsum": None}
                    for jt in range(NJ):
                        units.append((st, jt))

            ets = {}
            norm_due = {}
            for k in range(len(units) + DEPTH + 3):
                if k < len(units):
                    st, jt = units[k]
                    ets[k] = (st, jt, emit_unit_s(st, jt))
                for p in pieces_at.pop(k, ()):
                    emit_piece(p)
                if k in norm_due:
                    st_n = norm_due.pop(k)
                    emit_normalize(st_n, final=(st_n is units[-1][0]))
                j = k - DEPTH
                if 0 <= j < len(units):
                    st, jt, et = ets.pop(j)
                    emit_consumers(st, jt, et)
                    if jt == NJ - 1:
                        norm_due[k + 1] = st

    nc.compile()
    _CACHE[key] = nc
    return nc


def _prep_inputs(fmap, W_qkv, pos_f, pos_h, pos_w):
    """Host-side shard + layout prep. Returns per-core input maps."""
    bf = ml_dtypes.bfloat16
    inner = HEADS * D

    WqkT = W_qkv[:2 * inner].T.copy().astype(np.float32)    # [256, 1024]
    WqkT[:, :inner] *= SCALE                                # fold q scale
    WqkT_r = WqkT.reshape(CC, 128, 2 * inner)
    blocks = []
    for dest, h in BLK:
        col0 = (0 if dest == "q" else inner) + h * 128
        for cc in range(CC):
            blocks.append(WqkT_r[cc][:, col0:col0 + 128])
    wqk_host = np.ascontiguousarray(np.concatenate(blocks, axis=1)).astype(bf)

    WvT = W_qkv[2 * inner:].T.copy().astype(np.float32)     # [256, 512]
    wv_host = np.ascontiguousarray(
        WvT.reshape(CC, 128, 512).transpose(1, 0, 2).reshape(128, CC * 512)).astype(bf)
    pos_host = np.ascontiguousarray(np.concatenate(
        [pos_f.T, pos_h.T, pos_w.T], axis=1)).astype(np.float32)  # [128, 36]

    in_maps = []
    for b in range(B):
        xb = fmap[b].reshape(C, N).astype(np.float32).reshape(CC, 128, N)
        xcols = []
        for xc0, xcw in XSPL:
            for cc in range(CC):
                xcols.append(xb[cc][:, xc0:xc0 + xcw])
        x_host = np.ascontiguousarray(np.concatenate(xcols, axis=1)).astype(bf)
        in_maps.append({"x": x_host, "wqk": wqk_host, "wv": wv_host,
                        "pos": pos_host})
    return in_maps


def kernel(fmap, W_qkv, pos_f, pos_h, pos_w):
    from concourse.bass_utils import run_bass_kernel_spmd

    nc = _build()
    in_maps = _prep_inputs(np.asarray(fmap), np.asarray(W_qkv), np.asarray(pos_f),
                           np.asarray(pos_h), np.asarray(pos_w))
    res = run_bass_kernel_spmd(nc, in_maps, core_ids=list(range(N_CORES)))
    out = np.stack([res.results[b]["out"].reshape(HEADS * D, F, HH, WW)
                    for b in range(B)])
    return out.astype(np.float32)


def benchmark(n_iters=30, reps=1):
    """Estimate per-execution device time by timing n_iters async dispatches.
    With reps>1 the NEFF repeats the whole kernel body reps times in a
    hardware For_i loop, amortizing dispatch overhead for timing."""
    import time
    import jax
    from jax.sharding import Mesh, PartitionSpec
    from jax.experimental.shard_map import shard_map
    import concourse.mybir as mybir
    from concourse import bass2jax

    nc = _build(reps)
    rng = np.random.default_rng(0)
    fmap = rng.standard_normal((B, C, F, HH, WW), dtype=np.float32)
    W = rng.standard_normal((3 * HEADS * D, C), dtype=np.float32) * C ** -0.5
    pf = rng.standard_normal((F, D), dtype=np.float32)
    ph = rng.standard_normal((HH, D), dtype=np.float32)
    pw = rng.standard_normal((WW, D), dtype=np.float32)
    in_maps = _prep_inputs(fmap, W, pf, ph, pw)

    bass2jax.install_neuronx_cc_hook()
    partition_name = nc.partition_id_tensor.name if nc.partition_id_tensor else None
    in_names, out_names, out_avals = [], [], []
    for alloc in nc.m.functions[0].allocations:
        if not isinstance(alloc, mybir.MemoryLocationSet):
            continue
        name = alloc.memorylocations[0].name
        if alloc.kind == "ExternalInput":
            if name != partition_name:
                in_names.append(name)
        elif alloc.kind == "ExternalOutput":
            out_names.append(name)
            out_avals.append(jax.core.ShapedArray(
                tuple(alloc.tensor_shape), mybir.dt.np(alloc.dtype)))
    n_params = len(in_names)
    zero_outs = [np.zeros(a.shape, a.dtype) for a in out_avals]
    all_in_names = in_names + out_names
    if partition_name is not None:
        all_in_names = all_in_names + [partition_name]

    def _body(*args):
        operands = list(args)
        if partition_name is not None:
            operands.append(bass2jax.partition_id_tensor())
        outs = bass2jax._bass_exec_p.bind(
            *operands, out_avals=tuple(out_avals), in_names=tuple(all_in_names),
            out_names=tuple(out_names), lowering_input_output_aliases=(),
            sim_require_finite=True, sim_require_nnan=True, nc=nc)
        return tuple(outs)

    devices = jax.devices()[:N_CORES]
    mesh = Mesh(np.asarray(devices), ("core",))
    specs = (PartitionSpec("core"),) * (n_params + len(out_names))
    fn = jax.jit(shard_map(_body, mesh=mesh, in_specs=specs,
                           out_specs=(PartitionSpec("core"),) * len(out_names),
                           check_rep=False))
    concat_in = [np.concatenate([in_maps[c][k] for c in range(N_CORES)], axis=0)
                 for k in in_names]
    concat_zero = [np.zeros((N_CORES * z.shape[0], *z.shape[1:]), z.dtype)
                   for z in zero_outs]
    args = [jax.device_put(a) for a in concat_in + concat_zero]

    outs = fn(*args)
    jax.block_until_ready(outs)
    # timed loop: async dispatch, single final block
    t0 = time.perf_counter()
    for _ in range(n_iters):
        outs = fn(*args)
    jax.block_until_ready(outs)
    t1 = time.perf_counter()
    per_run_ns = (t1 - t0) / n_iters * 1e9
    return per_run_ns


if __name__ == "__main__":
    ns = benchmark()
    print(f"HW exec time: {ns:.0f} ns")

